# revision 1
# baseline (speedup 1.0000x reference)
"""Trainium2 Bass kernel for nn_CSCLoss: multi-scale bilinear point-sampling
cosine-consistency loss.

loss = 1 - mean_{pairs,(b,n)} <normalize(sample(feat_i, p_bn)), normalize(sample(feat_j, p_bn))>

Sharding: data-parallel over batch — 32 images -> 8 cores x 4 images; the
host sums the 8 per-core partial sums and applies the 1 - total/count
epilogue (the all-reduce of the sharding hint, done on 8 scalars).

Per-core dataflow (dense, HBM-bandwidth-bound):
 - All per-point scalar math (pixel coords, floor, lerp weights, gather
   indices) runs on partition 0 in [1,128]-wide vector ops from `boxes`.
 - Gather indices are laid out in ap_gather's wrapped format and replicated
   to all 8 DVE 16-partition groups with a 0-stride DRAM->SBUF DMA; bilinear
   weights are replicated to all 128 partitions the same way.
 - Feature maps stream through SBUF as multi-image [128ch, nb*H*W] tiles
   (21 MiB/core at DMA line rate — the roofline) split over two HWDGE rings;
   gpsimd.ap_gather (batched — each dispatch has ~4us fixed cost) extracts
   the 4 bilinear corners per point, DVE applies the lerp weights and
   reduces to sampled vectors v[c, col], col = b*32 + s*4 + rb.
 - Channel reductions (squared norms, pairwise dots) are ones-vector
   matmuls on PE accumulating the two 128-channel chunks into PSUM [1,128].
 - The cosine epilogue runs on partition 0 and emits one [1,1] partial.
"""

import sys
from contextlib import ExitStack

import numpy as np

if "/opt/trn_rl_repo" not in sys.path:
    sys.path.insert(0, "/opt/trn_rl_repo")

B, N, C = 32, 32, 256
LEVELS = [(64, 64), (32, 32), (16, 16)]  # (H, W)
NB = [1, 1, 1]                           # images per gather batch
LORDER = [2, 1, 0]                       # small levels first (early Pool start)
N_CORES = 8
BL = B // N_CORES          # images per core
NPTS = BL * N              # 128 points per core
PAIRS = [(0, 1), (0, 2), (1, 2)]
EPS = 1e-12

_CACHE = {}


def _build_program():
    from concourse import bacc, bass, mybir, tile, library_config

    dt = mybir.dt
    AL = mybir.AluOpType

    nc = bacc.Bacc("TRN2", target_bir_lowering=False, debug=False)

    feats = [
        nc.dram_tensor(f"feat{i}", [BL, C, H, W], dt.float32, kind="ExternalInput")
        for i, (H, W) in enumerate(LEVELS)
    ]
    boxes = nc.dram_tensor("boxes", [BL, N, 4], dt.float32, kind="ExternalInput")
    out = nc.dram_tensor("out", [1, 1], dt.float32, kind="ExternalOutput")

    with tile.TileContext(nc) as tc, ExitStack() as ctx:
        pool = ctx.enter_context(tc.tile_pool(name="sbuf", bufs=1))
        pa = ctx.enter_context(tc.tile_pool(name="pa", bufs=1))
        pstream = ctx.enter_context(tc.tile_pool(name="stream", bufs=1))
        pwork = ctx.enter_context(tc.tile_pool(name="work", bufs=2))
        ppsum = ctx.enter_context(tc.tile_pool(name="psum", bufs=1, space="PSUM"))
        pdram = ctx.enter_context(tc.tile_pool(name="dram", bufs=1, space="DRAM"))

        nc.gpsimd.load_library(library_config.ap_gather)

        # constants for PE-based broadcasts
        ones1 = pool.tile([1, 128], dt.float32)
        nc.vector.memset(ones1[:], 1.0)
        # REPLf[k, q] = 1.0 iff q % 16 == k  (block-replicate [16,*] -> [128,*])
        repl_i = pool.tile([16, 128], dt.int32)
        nc.gpsimd.iota(repl_i[:], pattern=[[1, 128]], base=0, channel_multiplier=15)
        nc.vector.tensor_scalar(
            out=repl_i[:], in0=repl_i[:], scalar1=15, scalar2=None,
            op0=AL.bitwise_and,
        )
        replf = pool.tile([16, 128], dt.float32)
        nc.vector.tensor_scalar(
            out=replf[:], in0=repl_i[:], scalar1=0, scalar2=None, op0=AL.is_equal,
        )

        # ---- boxes load first on the scalar ring (phase A needs it) ----
        bxr = pool.tile([1, BL * N * 4], dt.float32)  # [1, 512] flat boxes
        nc.scalar.dma_start(
            out=bxr[:].rearrange("o (a f) -> o a f", a=BL),
            in_=boxes.rearrange("b n c -> b (n c)"),
        )

        # ---- feature-map streaming DMAs, issued up front ----
        # small levels first on the scalar ring (their gathers start the Pool
        # pipeline early); lvl0 on the sync ring.
        dma_eng = [nc.sync, nc.scalar, nc.scalar]
        T_tiles = {}
        for li in LORDER:
            H, W = LEVELS[li]
            HW = H * W
            nb = NB[li]
            fview = feats[li].rearrange("b c h w -> c b (h w)")
            SBUFS = [5, 8, 8]
            for u in range(BL // nb):
                for ch in range(2):
                    T = pstream.tile(
                        [128, nb * HW], dt.float32, name=f"T{li}_{u}_{ch}",
                        tag=f"T{li}", bufs=SBUFS[li],
                    )
                    dma_eng[li].dma_start(
                        out=T[:].rearrange("c (b q) -> c b q", b=nb),
                        in_=fview[ch * 128:(ch + 1) * 128, u * nb:(u + 1) * nb, :],
                    )
                    T_tiles[(li, u, ch)] = T

        # ---- Phase A: per-point scalar math on partition 0 (DVE) ----
        bxv = bxr[:].rearrange("o (j c) -> o j c", c=4)
        cx = bxv[:, :, 0]  # [1, 128] stride 4
        cy = bxv[:, :, 1]

        def axis_prep(coord, E, ax):
            """pixel coord p=clip(c*(E-1),0,E-1); e0=clamp(floor(p),0,E-2);
            w=p-e0. floor via 16.16 fixed point (exact *2^16; conversion
            error <=2^-16 absorbed by the lerp weight)."""
            pf = pa.tile([1, NPTS], dt.float32, name=f"pf{ax}", tag=f"pf{ax}")
            nc.vector.tensor_scalar(
                out=pf[:], in0=coord, scalar1=float(E - 1), scalar2=0.0,
                op0=AL.mult, op1=AL.max,
            )
            nc.vector.tensor_scalar_min(out=pf[:], in0=pf[:], scalar1=float(E - 1))
            pxs = pa.tile([1, NPTS], dt.float32, name=f"pxs{ax}", tag=f"pxs{ax}")
            nc.vector.tensor_scalar(
                out=pxs[:], in0=pf[:], scalar1=65536.0, scalar2=None, op0=AL.mult,
            )
            ifx = pa.tile([1, NPTS], dt.int32, name=f"ifx{ax}", tag=f"ifx{ax}")
            nc.vector.tensor_copy(out=ifx[:], in_=pxs[:])
            x0i = pa.tile([1, NPTS], dt.int32, name=f"x0i{ax}", tag=f"x0i{ax}")
            nc.vector.tensor_scalar(
                out=x0i[:], in0=ifx[:], scalar1=16, scalar2=None,
                op0=AL.arith_shift_right,
            )
            e0 = pa.tile([1, NPTS], dt.float32, name=f"e0{ax}", tag=f"e0{ax}")
            nc.vector.tensor_copy(out=e0[:], in_=x0i[:])
            nc.vector.tensor_scalar_min(out=e0[:], in0=e0[:], scalar1=float(E - 2))
            we = pa.tile([1, NPTS], dt.float32, name=f"we{ax}", tag=f"we{ax}")
            nc.vector.tensor_tensor(out=we[:], in0=pf[:], in1=e0[:], op=AL.subtract)
            return e0, we

        V = [
            [pool.tile([128, NPTS], dt.float32, name=f"V{li}_{ch}") for ch in range(2)]
            for li in range(3)
        ]
        for li in LORDER:
            H, W = LEVELS[li]
            HW = H * W
            nb = NB[li]
            x0f, wx = axis_prep(cx, W, "x")
            y0f, wy = axis_prep(cy, H, "y")

            # basefu[point(b,n)] = y0*W + x0 + (b % nb)*HW  (unit-local image
            # offset folded in; values < nb*HW <= 16384 fit int16)
            basef = pa.tile([1, NPTS], dt.float32, name="basef", tag="basef")
            nc.vector.tensor_scalar(
                out=basef[:], in0=y0f[:], scalar1=float(W), scalar2=None,
                op0=AL.mult,
            )
            nc.vector.tensor_tensor(
                out=basef[:], in0=basef[:], in1=x0f[:], op=AL.add
            )
            basef_b = basef[:].rearrange("o (b n) -> o b n", b=BL)
            for b in range(BL):
                off = float((b % nb) * HW)
                if off:
                    nc.vector.tensor_scalar(
                        out=basef_b[:, b], in0=basef_b[:, b],
                        scalar1=off, scalar2=None, op0=AL.add,
                    )

            # wrapped index row: flat layout r*32 + b*8 + s, r=rb*4+k,
            # value = basefu[point(b, 4s+rb)] + dk(k), dk = (k//2)*W + k%2
            srow = pa.tile([1, 16 * 32], dt.float32, name="srow", tag="srow")
            srow_v = srow[:].rearrange("o (r b s) -> o r b s", r=16, b=BL)
            basef_v = basef[:].rearrange("o (b s f) -> o b s f", b=BL, f=4)
            for rb in range(4):
                for k in range(4):
                    dk = float((k // 2) * W + (k % 2))
                    nc.vector.tensor_scalar(
                        out=srow_v[:, rb * 4 + k],
                        in0=basef_v[:, :, :, rb],
                        scalar1=dk, scalar2=None, op0=AL.add,
                    )
            sidx = pdram.tile([16, 32], dt.float32, name=f"sidx{li}")
            nc.gpsimd.dma_start(
                out=sidx[:], in_=srow[:].rearrange("o (r c) -> o r c", r=16),
            )
            s16f = pa.tile([16, 32], dt.float32, name="s16f", tag="s16f")
            nc.gpsimd.dma_start(out=s16f[:], in_=sidx[:])
            widx_ps = ppsum.tile([128, 32], dt.float32, name=f"widxps{li}", tag="widxps")
            nc.tensor.matmul(
                widx_ps[:], replf[:], s16f[:], start=True, stop=True,
            )
            widx = pool.tile([128, 32], dt.int16, name=f"widx{li}")
            nc.vector.tensor_copy(out=widx[:], in_=widx_ps[:])

            # corner weights, k = yi*2 + xi, packed k-major then reordered to
            # the gather-output column order (b, s, rb, k)
            w1x = pa.tile([1, NPTS], dt.float32, name="w1x", tag="w1x")
            nc.vector.tensor_scalar(
                out=w1x[:], in0=wx[:], scalar1=-1.0, scalar2=1.0,
                op0=AL.mult, op1=AL.add,
            )
            w1y = pa.tile([1, NPTS], dt.float32, name="w1y", tag="w1y")
            nc.vector.tensor_scalar(
                out=w1y[:], in0=wy[:], scalar1=-1.0, scalar2=1.0,
                op0=AL.mult, op1=AL.add,
            )
            wkt = pa.tile([1, 4 * NPTS], dt.float32, name="wkt", tag="wkt")
            for k, (wyt, wxt) in enumerate(
                [(w1y, w1x), (w1y, wx), (wy, w1x), (wy, wx)]
            ):
                nc.vector.tensor_tensor(
                    out=wkt[:, k * NPTS:(k + 1) * NPTS],
                    in0=wyt[:], in1=wxt[:], op=AL.mult,
                )
            wrow = pa.tile([1, NPTS * 4], dt.float32, name="wrow", tag="wrow")
            # wrow col = b*128 + s*16 + rb*4 + k <- wkt[k*128 + b*32 + s*4 + rb]
            wkt_v = wkt[:].rearrange(
                "o (k b s rb) -> o k b s rb", k=4, b=BL, s=8
            )
            wrow_v = wrow[:].rearrange(
                "o (b s rb k) -> o b s rb k", b=BL, s=8, rb=4
            )
            for b in range(BL):
                nc.vector.tensor_copy(
                    out=wrow_v[:, b],
                    in_=wkt_v[:, :, b].rearrange("o k s rb -> o s rb k"),
                )
            wb_ps = ppsum.tile([128, NPTS * 4], dt.float32, name=f"wbps{li}", tag="wbps")
            nc.tensor.matmul(wb_ps[:], ones1[:], wrow[:], start=True, stop=True)
            wb = pool.tile([128, NPTS * 4], dt.float32, name=f"wb{li}")
            nc.vector.tensor_copy(out=wb[:], in_=wb_ps[:])
            # ---- this level's gathers + lerp (V col = b*32 + s*4 + rb) ----
            ncols = nb * 128
            for u in range(BL // nb):
                for ch in range(2):
                    T = T_tiles[(li, u, ch)]
                    og = pwork.tile(
                        [128, ncols], dt.float32, name=f"og{li}", tag="og"
                    )
                    nc.gpsimd.ap_gather(
                        out_ap=og[:], in_ap=T[:],
                        idxs_ap=widx[:, u * nb * 8:(u + 1) * nb * 8],
                        channels=128, num_elems=nb * HW, d=1, num_idxs=ncols,
                    )
                    nc.vector.tensor_tensor(
                        out=og[:], in0=og[:],
                        in1=wb[:, u * ncols:(u + 1) * ncols], op=AL.mult,
                    )
                    nc.vector.tensor_reduce(
                        out=V[li][ch][:, u * nb * 32:(u + 1) * nb * 32],
                        in_=og[:].rearrange("c (n f) -> c n f", f=4),
                        axis=mybir.AxisListType.X, op=AL.add,
                    )

        # ---- Phase C: channel reductions via ones-matmul into PSUM ----
        ones = pool.tile([128, 1], dt.float32)
        nc.vector.memset(ones[:], 1.0)

        def colsum(name, make_in):
            ps = ppsum.tile([1, NPTS], dt.float32, name=name)
            for ch in range(2):
                prod = pwork.tile(
                    [128, NPTS], dt.float32, name=f"prod{name}{ch}", tag="prod"
                )
                make_in(prod, ch)
                nc.tensor.matmul(
                    ps[:], ones[:], prod[:], start=(ch == 0), stop=(ch == 1),
                )
            sb = pool.tile([1, NPTS], dt.float32, name=f"sb{name}")
            nc.vector.tensor_copy(out=sb[:], in_=ps[:])
            return sb

        ss = [
            colsum(
                f"ss{li}",
                lambda prod, ch, li=li: nc.vector.tensor_tensor(
                    out=prod[:], in0=V[li][ch][:], in1=V[li][ch][:], op=AL.mult
                ),
            )
            for li in range(3)
        ]
        dots = {}
        for i, j in PAIRS:
            dots[(i, j)] = colsum(
                f"d{i}{j}",
                lambda prod, ch, i=i, j=j: nc.vector.tensor_tensor(
                    out=prod[:], in0=V[i][ch][:], in1=V[j][ch][:], op=AL.mult
                ),
            )

        # ---- Phase D: cosine epilogue on partition 0 ----
        rns = []
        for li in range(3):
            nrm = pool.tile([1, NPTS], dt.float32, name=f"nrm{li}")
            nc.scalar.sqrt(out=nrm[:], in_=ss[li][:])
            nc.vector.tensor_scalar_max(out=nrm[:], in0=nrm[:], scalar1=EPS)
            rn = pool.tile([1, NPTS], dt.float32, name=f"rn{li}")
            nc.vector.reciprocal(out=rn[:], in_=nrm[:])
            rns.append(rn)

        tot = pool.tile([1, NPTS], dt.float32)
        first = True
        for i, j in PAIRS:
            t = pool.tile([1, NPTS], dt.float32, name=f"t{i}{j}")
            nc.vector.tensor_tensor(
                out=t[:], in0=dots[(i, j)][:], in1=rns[i][:], op=AL.mult
            )
            nc.vector.tensor_tensor(out=t[:], in0=t[:], in1=rns[j][:], op=AL.mult)
            if first:
                nc.vector.tensor_copy(out=tot[:], in_=t[:])
                first = False
            else:
                nc.vector.tensor_tensor(out=tot[:], in0=tot[:], in1=t[:], op=AL.add)

        res = pool.tile([1, 1], dt.float32)
        nc.vector.tensor_reduce(
            out=res[:], in_=tot[:], axis=mybir.AxisListType.X, op=AL.add
        )
        nc.sync.dma_start(out=out.ap(), in_=res[:])

    nc.compile()
    return nc


def _get_program():
    if "nc" not in _CACHE:
        _CACHE["nc"] = _build_program()
    return _CACHE["nc"]


def _run_device(feat0, feat1, feat2, boxes, **run_kwargs):
    """Shard inputs batch-wise over the 8 cores, run the SPMD program, and
    return the BassKernelResults (one {"out": [1,1]} per core)."""
    from concourse.bass_utils import run_bass_kernel_spmd

    nc = _get_program()

    feats = [
        np.ascontiguousarray(np.asarray(f, dtype=np.float32))
        for f in (feat0, feat1, feat2)
    ]
    boxes = np.ascontiguousarray(np.asarray(boxes, dtype=np.float32))

    in_maps = []
    for k in range(N_CORES):
        sl = slice(k * BL, (k + 1) * BL)
        in_maps.append(
            {
                "feat0": feats[0][sl],
                "feat1": feats[1][sl],
                "feat2": feats[2][sl],
                "boxes": boxes[sl],
            }
        )

    return run_bass_kernel_spmd(
        nc, in_maps, core_ids=list(range(N_CORES)), **run_kwargs
    )


def kernel(feat0, feat1, feat2, boxes):
    r = _run_device(feat0, feat1, feat2, boxes)
    total = np.float64(0.0)
    for m in r.results:
        total += np.float64(m["out"].reshape(-1)[0])

    count = B * N * len(PAIRS)
    avg = np.float32(total) / np.float32(count)
    loss = np.float32(1.0) - avg
    loss = np.nan_to_num(loss, nan=0.0, posinf=1.0, neginf=0.0)
    return np.array(np.clip(loss, 0.0, 2.0), dtype=np.float32)



# revision 2
# speedup vs baseline: 1.1861x; 1.1861x over previous
"""Trainium2 Bass kernel for nn_CSCLoss: multi-scale bilinear point-sampling
cosine-consistency loss.

loss = 1 - mean_{pairs,(b,n)} <normalize(sample(feat_i, p_bn)), normalize(sample(feat_j, p_bn))>

Sharding: data-parallel over batch - 32 images -> 8 cores x 4 images; the
host sums the 8 per-core partial sums and applies the 1 - total/count
epilogue.

Per-core dataflow (HBM-streaming-bound, ~22 MB/core):
 - Feature maps stream HBM->SBUF in 6 big tiles (level2 and level1 batch all
   4 images x both 128-channel chunks in one tile; level0 is 4 per-image
   tiles, also holding both channel chunks side by side), split across the
   two HWDGE rings (sync=chunk0, scalar=chunk1).
 - Exactly 6 gpsimd.ap_gather dispatches (one per tile) extract the 4
   bilinear corners for every point - the ~4us fixed dispatch cost of each
   gpsimd custom op is what dominated the previous version (24 dispatches).
 - All per-point scalar math runs on partition 0 in [1,384]-wide DVE ops
   (3 levels x 128 points at once); gather indices are built directly in
   int16 and replicated to the 8 DVE 16-partition groups with a 0-stride
   DRAM->SBUF broadcast DMA; lerp weights are replicated to 128 partitions
   the same way.
 - DVE applies lerp weights and reduces corners to V[c, col]; channel
   reductions (squared norms, pairwise dots) are ones-vector matmuls into
   PSUM, emitted per-chunk as soon as each V slice lands so only the last
   image's work sits in the tail.
 - Cosine epilogue on partition 0: one sqrt + one reciprocal on [1,384].
"""

import sys
from contextlib import ExitStack

import numpy as np

if "/opt/trn_rl_repo" not in sys.path:
    sys.path.insert(0, "/opt/trn_rl_repo")

B, N, C = 32, 32, 256
LEVELS = [(64, 64), (32, 32), (16, 16)]  # (H, W)
N_CORES = 8
BL = B // N_CORES          # images per core
NPTS = BL * N              # 128 points per core
PAIRS = [(0, 1), (0, 2), (1, 2)]
EPS = 1e-12

_CACHE = {}


def _build_program():
    from concourse import bacc, bass, mybir, tile, library_config

    dt = mybir.dt
    AL = mybir.AluOpType
    F32 = dt.float32

    nc = bacc.Bacc("TRN2", target_bir_lowering=False, debug=False)

    feats = [
        nc.dram_tensor(f"feat{i}", [BL, C, H, W], F32, kind="ExternalInput")
        for i, (H, W) in enumerate(LEVELS)
    ]
    boxes = nc.dram_tensor("boxes", [BL, N, 4], F32, kind="ExternalInput")
    out = nc.dram_tensor("out", [1, 1], F32, kind="ExternalOutput")

    with tile.TileContext(nc) as tc, ExitStack() as ctx:
        pool = ctx.enter_context(tc.tile_pool(name="sbuf", bufs=1))
        pa = ctx.enter_context(tc.tile_pool(name="pa", bufs=1))
        pstream = ctx.enter_context(tc.tile_pool(name="stream", bufs=1))
        pwork = ctx.enter_context(tc.tile_pool(name="work", bufs=2))
        ppsum = ctx.enter_context(tc.tile_pool(name="psum", bufs=1, space="PSUM"))
        pdram = ctx.enter_context(tc.tile_pool(name="dram", bufs=1, space="DRAM"))

        nc.gpsimd.load_library(library_config.ap_gather)

        # ---- boxes load first on the sync ring ----
        bxr = pool.tile([1, BL * N * 4], F32)  # [1, 512] flat boxes
        nc.sync.dma_start(
            out=bxr[:].rearrange("o (a f) -> o a f", a=BL),
            in_=boxes.rearrange("b n c -> b (n c)"),
        )

        # ---- feature-map streaming: 6 tiles, both channel chunks per tile ----
        #   T[:, 0:half]     = channels 0..127   (sec 0, sync ring)
        #   T[:, half:2half] = channels 128..255 (sec 1, scalar ring)
        T2 = pstream.tile([128, 2048], F32, name="T2")   # l2, nb=4,  8 KB/part
        T1 = pstream.tile([128, 8192], F32, name="T1")   # l1, nb=4, 32 KB/part
        T0 = [
            pstream.tile([128, 8192], F32, name=f"T0_{u}", tag="T0", bufs=2)
            for u in range(BL)
        ]                                                # l0 per-img, 64 KB/part

        def stream(T, li, half, u=None):
            fv = feats[li].rearrange("b c h w -> c b (h w)")
            for sec, eng in ((0, nc.sync), (1, nc.scalar)):
                cs = slice(sec * 128, sec * 128 + 128)
                if u is None:  # all 4 images
                    eng.dma_start(
                        out=T[:, sec * half:(sec + 1) * half].rearrange(
                            "c (b q) -> c b q", b=BL
                        ),
                        in_=fv[cs, :, :],
                    )
                else:
                    eng.dma_start(
                        out=T[:, sec * half:(sec + 1) * half],
                        in_=fv[cs, u, :],
                    )

        stream(T2, 2, 1024)
        stream(T1, 1, 4096)
        # NOTE: T0 streams are issued AFTER the index/weight staging DMAs so
        # the staging is not FIFO-queued behind 17 MB on the scalar ring.

        # ---- Phase A: per-point scalar math on partition 0 (DVE) ----
        # segment layout on [1, 384]: cols li*128 + (b*32 + n), n = s*4 + rb
        SEG = lambda t, li: t[:, li * 128:(li + 1) * 128]

        # constants (boxes-independent, built in the DMA shadow)
        CS = pa.tile([1, 384], F32, name="CS")       # E-1 per level
        for li, (H, W) in enumerate(LEVELS):
            nc.vector.memset(SEG(CS, li), float(W - 1))
        CE2 = pa.tile([1, 384], F32, name="CE2")     # E-2
        nc.vector.tensor_scalar(
            out=CE2[:], in0=CS[:], scalar1=-1.0, scalar2=None, op0=AL.add
        )
        CW = pa.tile([1, 384], F32, name="CW")       # E (row stride)
        nc.vector.tensor_scalar(
            out=CW[:], in0=CS[:], scalar1=1.0, scalar2=None, op0=AL.add
        )
        OFF = pa.tile([1, 384], F32, name="OFF")     # (b % nb) * HW offsets
        nc.vector.memset(SEG(OFF, 0), 0.0)           # l0: nb=1
        for li in (1, 2):
            HW = LEVELS[li][0] * LEVELS[li][1]
            ov = SEG(OFF, li).rearrange("o (b n) -> o b n", b=BL)
            for b in range(BL):
                nc.vector.memset(ov[:, b], float(b * HW))

        bxv = bxr[:].rearrange("o (j c) -> o j c", c=4)

        def axis_prep(coord, ax):
            """pixel coord p=clip(c*(E-1),0,E-1); e0=clamp(floor(p),0,E-2);
            w=p-e0. floor via 16.16 fixed point (exact *2^16; conversion
            error <=2^-16 absorbed by the lerp weight)."""
            pf = pa.tile([1, 384], F32, name=f"pf{ax}")
            for li in range(3):
                nc.vector.tensor_tensor(
                    out=SEG(pf, li), in0=coord, in1=SEG(CS, li), op=AL.mult
                )
            nc.vector.tensor_scalar_max(out=pf[:], in0=pf[:], scalar1=0.0)
            nc.vector.tensor_tensor(out=pf[:], in0=pf[:], in1=CS[:], op=AL.min)
            pxs = pa.tile([1, 384], F32, name=f"pxs{ax}")
            nc.vector.tensor_scalar(
                out=pxs[:], in0=pf[:], scalar1=65536.0, scalar2=None, op0=AL.mult
            )
            ifx = pa.tile([1, 384], dt.int32, name=f"ifx{ax}")
            nc.vector.tensor_copy(out=ifx[:], in_=pxs[:])
            nc.vector.tensor_scalar(
                out=ifx[:], in0=ifx[:], scalar1=16, scalar2=None,
                op0=AL.arith_shift_right,
            )
            e0 = pa.tile([1, 384], F32, name=f"e0{ax}")
            nc.vector.tensor_copy(out=e0[:], in_=ifx[:])
            nc.vector.tensor_tensor(out=e0[:], in0=e0[:], in1=CE2[:], op=AL.min)
            we = pa.tile([1, 384], F32, name=f"we{ax}")
            nc.vector.tensor_tensor(out=we[:], in0=pf[:], in1=e0[:], op=AL.subtract)
            return e0, we

        e0x, wx = axis_prep(bxv[:, :, 0], "x")
        e0y, wy = axis_prep(bxv[:, :, 1], "y")

        # basef[li, b, n] = y0*W + x0 + (b%nb)*HW   (exact small integers)
        basef = pa.tile([1, 384], F32, name="basef")
        nc.vector.tensor_tensor(out=basef[:], in0=e0y[:], in1=CW[:], op=AL.mult)
        nc.vector.tensor_tensor(out=basef[:], in0=basef[:], in1=e0x[:], op=AL.add)
        nc.vector.tensor_tensor(out=basef[:], in0=basef[:], in1=OFF[:], op=AL.add)
        basei = pa.tile([1, 384], dt.int16, name="basei")
        nc.vector.tensor_copy(out=basei[:], in_=basef[:])

        # ---- srow: wrapped gather indices, int16, [16 rows, 192 q-cols] ----
        # gather out col j = q*16 + r with the index stored at wrapped [r, q];
        # row r = rb*4 + k.  q-col sections:
        #   l2: q0=0,   q = sec*32 + b*8 + s   (og col = sec*512+b*128+s*16+rb*4+k)
        #   l1: q0=64,  same
        #   l0: q0=128, q = u*16 + sec*8 + s   (og_u col = sec*128+s*16+rb*4+k)
        # value = basei[li, b, n=s*4+rb] + dk(k) + sec*(nb*HW)
        srow = pa.tile([1, 16 * 192], dt.int16, name="srow")
        srk = srow[:].rearrange("o (rb k q) -> o rb k q", rb=4, k=4)
        srq = srow[:].rearrange("o (r q) -> o r q", r=16)

        for li, q0, nb in ((2, 0, 4), (1, 64, 4), (0, 128, 1)):
            H, W = LEVELS[li]
            HW = H * W
            # [o, rb, b, s] view of this level's base indices
            it = SEG(basei, li).rearrange(
                "o (b s f) -> o f b s", b=BL, f=4
            )
            for k in range(4):
                dk = (k // 2) * W + (k % 2)
                if li == 0:
                    ot = srk[:, :, k, q0:q0 + 64].rearrange(
                        "o rb (u sec s) -> o rb u sec s", u=BL, sec=2
                    )[:, :, :, 0]
                else:
                    ot = srk[:, :, k, q0:q0 + 32].rearrange(
                        "o rb (b s) -> o rb b s", b=BL
                    )
                nc.vector.tensor_scalar(
                    out=ot, in0=it, scalar1=dk, scalar2=None, op0=AL.add
                )
            # sec=1 columns = sec=0 columns + nb*HW
            if li == 0:
                sv = srq[:, :, q0:q0 + 64].rearrange(
                    "o r (u sec s) -> o r u sec s", u=BL, sec=2
                )
                nc.vector.tensor_scalar(
                    out=sv[:, :, :, 1], in0=sv[:, :, :, 0],
                    scalar1=nb * HW, scalar2=None, op0=AL.add,
                )
            else:
                sv = srq[:, :, q0:q0 + 64].rearrange(
                    "o r (sec b s) -> o r sec b s", sec=2, b=BL
                )
                nc.vector.tensor_scalar(
                    out=sv[:, :, 1], in0=sv[:, :, 0],
                    scalar1=nb * HW, scalar2=None, op0=AL.add,
                )

        # ---- replicate srow -> widx [128, 192] via DRAM round trip ----
        sidx_d = pdram.tile([16, 192], dt.int16, name="sidx")
        nc.scalar.dma_start(out=sidx_d[:], in_=srq)
        widx = pool.tile([128, 192], dt.int16, name="widx")
        nc.scalar.dma_start(
            out=widx[:],
            in_=sidx_d[:].unsqueeze(0).broadcast_to([8, 16, 192]),
        )

        # ---- lerp corner weights wrow [1, 1536] -> wb [128, 1536] ----
        # sections: l2: 0:512 (b,s,rb,k), l1: 512:1024, l0: 1024+128u (s,rb,k)
        w1x = pa.tile([1, 384], F32, name="w1x")
        nc.vector.tensor_scalar(
            out=w1x[:], in0=wx[:], scalar1=-1.0, scalar2=1.0,
            op0=AL.mult, op1=AL.add,
        )
        w1y = pa.tile([1, 384], F32, name="w1y")
        nc.vector.tensor_scalar(
            out=w1y[:], in0=wy[:], scalar1=-1.0, scalar2=1.0,
            op0=AL.mult, op1=AL.add,
        )
        wrow = pa.tile([1, 1536], F32, name="wrow")
        for li, w0 in ((2, 0), (1, 512), (0, 1024)):
            iv = lambda t, li=li: SEG(t, li).rearrange(
                "o (b s f) -> o b s f", b=BL, f=4
            )
            wseg = wrow[:, w0:w0 + 512].rearrange(
                "o (b s f k) -> o b s f k", b=BL, s=8, f=4
            )
            for k, (wyt, wxt) in enumerate(
                [(w1y, w1x), (w1y, wx), (wy, w1x), (wy, wx)]
            ):
                nc.vector.tensor_tensor(
                    out=wseg[:, :, :, :, k], in0=iv(wyt), in1=iv(wxt),
                    op=AL.mult,
                )
        wrow_d = pdram.tile([1, 1536], F32, name="wrow_d")
        nc.scalar.dma_start(out=wrow_d[:], in_=wrow[:])
        wb = pool.tile([128, 1536], F32, name="wb")
        nc.scalar.dma_start(out=wb[:], in_=wrow_d[:].broadcast_to([128, 1536]))

        # l0 streams issued now (scalar-ring FIFO: after the staging DMAs)
        for u in range(BL):
            stream(T0[u], 0, 4096, u=u)

        # ---- gathers + lerp + reduce + per-chunk channel sums ----
        V = pool.tile([128, 768], F32, name="V")
        ones = pool.tile([128, 1], F32)
        nc.vector.memset(ones[:], 1.0)

        ps_ss = ppsum.tile([1, 512], F32, name="ps_ss")    # ss2 | ss1
        ps_ss0 = ppsum.tile([1, 256], F32, name="ps_ss0")  # (u, sec, n)
        ps_d12 = ppsum.tile([1, 256], F32, name="ps_d12")
        ps_d01 = ppsum.tile([1, 256], F32, name="ps_d01")
        ps_d02 = ppsum.tile([1, 256], F32, name="ps_d02")

        def gather(T, nelems, q0, qw, w0, v0, tag):
            nidx = qw * 16
            og = pwork.tile([128, nidx], F32, name=f"og{tag}", tag=f"og{qw}")
            nc.gpsimd.ap_gather(
                out_ap=og[:], in_ap=T[:], idxs_ap=widx[:, q0:q0 + qw],
                channels=128, num_elems=nelems, d=1, num_idxs=nidx,
            )
            half = nidx // 2
            for sec in range(2):
                nc.vector.tensor_tensor(
                    out=og[:, sec * half:(sec + 1) * half],
                    in0=og[:, sec * half:(sec + 1) * half],
                    in1=wb[:, w0:w0 + half], op=AL.mult,
                )
            nv = nidx // 4
            nc.vector.tensor_reduce(
                out=V[:, v0:v0 + nv],
                in_=og[:].rearrange("c (n f) -> c n f", f=4),
                axis=mybir.AxisListType.X, op=AL.add,
            )

        def colsum(ps_slice, in0, in1, n, tag):
            prod = pwork.tile([128, n], F32, name=f"prod{tag}", tag=f"prod{n}")
            nc.vector.tensor_tensor(out=prod[:], in0=in0, in1=in1, op=AL.mult)
            nc.tensor.matmul(ps_slice, ones[:], prod[:], start=True, stop=True)

        # level 2 (V cols 0:256, layout (sec, b, n)), level 1 (256:512)
        gather(T2, 2048, 0, 64, 0, 0, "2")
        colsum(ps_ss[:, 0:256], V[:, 0:256], V[:, 0:256], 256, "ss2")
        gather(T1, 8192, 64, 64, 512, 256, "1")
        colsum(ps_ss[:, 256:512], V[:, 256:512], V[:, 256:512], 256, "ss1")
        colsum(ps_d12[:], V[:, 256:512], V[:, 0:256], 256, "d12")

        # level 0 units (V cols 512 + u*64, layout (sec, n))
        for u in range(BL):
            gather(T0[u], 8192, 128 + 16 * u, 16, 1024 + 128 * u,
                   512 + 64 * u, f"0{u}")
            v0u = V[:, 512 + 64 * u:512 + 64 * (u + 1)]
            v1u = V[:, 256:512].rearrange(
                "c (sec b n) -> c sec b n", sec=2, b=BL
            )[:, :, u, :]
            v2u = V[:, 0:256].rearrange(
                "c (sec b n) -> c sec b n", sec=2, b=BL
            )[:, :, u, :]
            sl = slice(u * 64, (u + 1) * 64)
            colsum(ps_ss0[:, sl], v0u, v0u, 64, f"ss0{u}")
            colsum(ps_d01[:, sl], v0u, v1u, 64, f"d01{u}")
            colsum(ps_d02[:, sl], v0u, v2u, 64, f"d02{u}")

        # ---- epilogue on partition 0 ----
        cs_ss = pool.tile([1, 512], F32, name="cs_ss")
        nc.vector.tensor_copy(out=cs_ss[:], in_=ps_ss[:])
        cs_ss0 = pool.tile([1, 256], F32, name="cs_ss0")
        nc.vector.tensor_copy(out=cs_ss0[:], in_=ps_ss0[:])
        cs_d01 = pool.tile([1, 256], F32, name="cs_d01")
        nc.vector.tensor_copy(out=cs_d01[:], in_=ps_d01[:])
        cs_d02 = pool.tile([1, 256], F32, name="cs_d02")
        nc.vector.tensor_copy(out=cs_d02[:], in_=ps_d02[:])
        cs_d12 = pool.tile([1, 256], F32, name="cs_d12")
        nc.vector.tensor_copy(out=cs_d12[:], in_=ps_d12[:])

        # channel-chunk sums -> ssc [1,384] (segs: l0, l1, l2 norms) and
        # dc [1,384] (segs: pairs (0,1), (0,2), (1,2)), in point order (b, n)
        ssc = pool.tile([1, 384], F32, name="ssc")
        dc = pool.tile([1, 384], F32, name="dc")

        def secsum(dst, src, l0_layout):
            if l0_layout:  # src [1, 256] cols (u, sec, n)
                v = src.rearrange("o (u sec n) -> o u sec n", u=BL, sec=2)
                nc.vector.tensor_tensor(
                    out=dst.rearrange("o (u n) -> o u n", u=BL),
                    in0=v[:, :, 0], in1=v[:, :, 1], op=AL.add,
                )
            else:  # src [1, 256] cols (sec, b, n)
                nc.vector.tensor_tensor(
                    out=dst, in0=src[:, 0:128], in1=src[:, 128:256], op=AL.add
                )

        secsum(SEG(ssc, 0), cs_ss0[:], True)
        secsum(SEG(ssc, 1), cs_ss[:, 256:512], False)
        secsum(SEG(ssc, 2), cs_ss[:, 0:256], False)
        secsum(SEG(dc, 0), cs_d01[:], True)
        secsum(SEG(dc, 1), cs_d02[:], True)
        secsum(SEG(dc, 2), cs_d12[:], False)

        # rn = 1 / max(sqrt(ssc), EPS) == 1 / sqrt(max(ssc, EPS^2))
        nc.vector.tensor_scalar_max(out=ssc[:], in0=ssc[:], scalar1=EPS * EPS)
        nrm = pool.tile([1, 384], F32, name="nrm")
        nc.scalar.sqrt(out=nrm[:], in_=ssc[:])
        rn = pool.tile([1, 384], F32, name="rn")
        nc.vector.reciprocal(out=rn[:], in_=nrm[:])

        rp = pool.tile([1, 384], F32, name="rp")
        for seg, (i, j) in enumerate(PAIRS):
            nc.vector.tensor_tensor(
                out=SEG(rp, seg), in0=SEG(rn, i), in1=SEG(rn, j), op=AL.mult
            )
        nc.vector.tensor_tensor(out=dc[:], in0=dc[:], in1=rp[:], op=AL.mult)
        res = pool.tile([1, 1], F32)
        nc.vector.tensor_reduce(
            out=res[:], in_=dc[:], axis=mybir.AxisListType.X, op=AL.add
        )
        nc.sync.dma_start(out=out.ap(), in_=res[:])

    nc.compile()
    return nc


def _get_program():
    if "nc" not in _CACHE:
        _CACHE["nc"] = _build_program()
    return _CACHE["nc"]


def _run_device(feat0, feat1, feat2, boxes, **run_kwargs):
    """Shard inputs batch-wise over the 8 cores, run the SPMD program, and
    return the BassKernelResults (one {"out": [1,1]} per core)."""
    from concourse.bass_utils import run_bass_kernel_spmd

    nc = _get_program()

    feats = [
        np.ascontiguousarray(np.asarray(f, dtype=np.float32))
        for f in (feat0, feat1, feat2)
    ]
    boxes = np.ascontiguousarray(np.asarray(boxes, dtype=np.float32))

    in_maps = []
    for k in range(N_CORES):
        sl = slice(k * BL, (k + 1) * BL)
        in_maps.append(
            {
                "feat0": feats[0][sl],
                "feat1": feats[1][sl],
                "feat2": feats[2][sl],
                "boxes": boxes[sl],
            }
        )

    return run_bass_kernel_spmd(
        nc, in_maps, core_ids=list(range(N_CORES)), **run_kwargs
    )


def kernel(feat0, feat1, feat2, boxes):
    r = _run_device(feat0, feat1, feat2, boxes)
    total = np.float64(0.0)
    for m in r.results:
        total += np.float64(m["out"].reshape(-1)[0])

    count = B * N * len(PAIRS)
    avg = np.float32(total) / np.float32(count)
    loss = np.float32(1.0) - avg
    loss = np.nan_to_num(loss, nan=0.0, posinf=1.0, neginf=0.0)
    return np.array(np.clip(loss, 0.0, 2.0), dtype=np.float32)


# revision 6
# speedup vs baseline: 1.3470x; 1.1357x over previous
"""Trainium2 Bass kernel for nn_CSCLoss: multi-scale bilinear point-sampling
cosine-consistency loss.

loss = 1 - mean_{pairs,(b,n)} <normalize(sample(feat_i, p_bn)), normalize(sample(feat_j, p_bn))>

Sharding: data-parallel over batch - 32 images -> 8 cores x 4 images; the
host sums the 8 per-core partial sums and applies the loss epilogue.

Per-core dataflow. ap_gather cost is ~27ns per INDEX nearly independent of
d (measured), so the design minimizes index count:
 - l2/l1 stream into tiles holding [A_s0|B_s0|A_s1|B_s1] where B = A shifted
   by one element (built by SBUF->SBUF DMA, no extra HBM). Any x-pair
   (p, p+1) is then an even-aligned d=2 block: of A if p is even, of B if p
   is odd. One gather index per (point, row, channel-chunk): 512 idx/level
   instead of 1024.
 - l0 (16.8 MB/core) keeps d=1 4-corner gathers (a B copy would cost too
   much SBUF/DMA): 8 per-(image,chunk) tiles of [128,4096], 1024 idx total.
 - All 12 stream DMAs ride the scalar HWDGE queue in arrival order; the
   sync queue carries only boxes, the 4 B-copies, index/weight staging and
   the result, so small transfers never sit behind megabytes of stream
   descriptors (the HW drains the two queues round-robin).
 - Index/weight math on partition 0 in wide DVE ops; int16 indices are
   replicated to the 8 gpsimd core groups by a 0-stride broadcast DMA,
   lerp weights to 128 partitions the same way.
 - Per-chunk channel sums (ones-matmul into PSUM) right after each V slice;
   final cosine epilogue is one sqrt + one reciprocal on [1,384].
"""

import sys
from contextlib import ExitStack

import numpy as np

if "/opt/trn_rl_repo" not in sys.path:
    sys.path.insert(0, "/opt/trn_rl_repo")

B, N, C = 32, 32, 256
LEVELS = [(64, 64), (32, 32), (16, 16)]  # (H, W)
N_CORES = 8
BL = B // N_CORES          # images per core
NPTS = BL * N              # 128 points per core
PAIRS = [(0, 1), (0, 2), (1, 2)]
EPS = 1e-12

_CACHE = {}


def _build_program():
    from concourse import bacc, bass, mybir, tile, library_config

    dt = mybir.dt
    AL = mybir.AluOpType
    F32 = dt.float32
    I16 = dt.int16

    nc = bacc.Bacc("TRN2", target_bir_lowering=False, debug=False)

    feats = [
        nc.dram_tensor(f"feat{i}", [BL, C, H, W], F32, kind="ExternalInput")
        for i, (H, W) in enumerate(LEVELS)
    ]
    boxes = nc.dram_tensor("boxes", [BL, N, 4], F32, kind="ExternalInput")
    out = nc.dram_tensor("out", [1, 1], F32, kind="ExternalOutput")

    with tile.TileContext(nc) as tc, ExitStack() as ctx:
        pool = ctx.enter_context(tc.tile_pool(name="sbuf", bufs=1))
        pa = ctx.enter_context(tc.tile_pool(name="pa", bufs=1))
        pstream = ctx.enter_context(tc.tile_pool(name="stream", bufs=1))
        pwork = ctx.enter_context(tc.tile_pool(name="work", bufs=2))
        ppsum = ctx.enter_context(tc.tile_pool(name="psum", bufs=1, space="PSUM"))
        pdram = ctx.enter_context(tc.tile_pool(name="dram", bufs=1, space="DRAM"))

        nc.gpsimd.load_library(library_config.ap_gather)

        # ---- boxes first on the sync queue ----
        bxr = pool.tile([1, BL * N * 4], F32)
        nc.sync.dma_start(
            out=bxr[:].rearrange("o (a f) -> o a f", a=BL),
            in_=boxes.rearrange("b n c -> b (n c)"),
        )

        # ---- stream tiles ----
        # l2/l1: [A_s0 | B_s0 | A_s1 | B_s1], each region nb*HW elements.
        T2 = pstream.tile([128, 4096], F32, name="T2")    # 16 KB/part
        T1 = pstream.tile([128, 16384], F32, name="T1")   # 64 KB/part
        # l0: per (image, chunk) tiles, 4-way rotating buffer (64 KB/part)
        T0 = [
            pstream.tile([128, 4096], F32, name=f"T0_{u}_{sec}", tag="T0",
                         bufs=4)
            for u in range(BL) for sec in range(2)
        ]

        # all A streams on the scalar queue, in gather order
        def fv(li):
            return feats[li].rearrange("b c h w -> c b (h w)")

        for li, T, n in ((2, T2, 1024), (1, T1, 4096)):
            for sec in range(2):
                nc.scalar.dma_start(
                    out=T[:, 2 * sec * n:(2 * sec + 1) * n].rearrange(
                        "c (b q) -> c b q", b=BL
                    ),
                    in_=fv(li)[sec * 128:sec * 128 + 128, :, :],
                )
        for u in range(BL):
            for sec in range(2):
                nc.scalar.dma_start(
                    out=T0[2 * u + sec][:],
                    in_=fv(0)[sec * 128:sec * 128 + 128, u, :],
                )

        # B copies (shift-by-one) on the sync queue, right after their A data;
        # the final element of each B region is never indexed - memset it so
        # the tile is fully initialized.
        for T, n in ((T2, 1024), (T1, 4096)):
            for sec in range(2):
                a0 = 2 * sec * n
                nc.sync.dma_start(
                    out=T[:, a0 + n:a0 + 2 * n - 1],
                    in_=T[:, a0 + 1:a0 + n],
                )
                nc.vector.memset(T[:, a0 + 2 * n - 1:a0 + 2 * n], 0.0)

        # ---- Phase A: per-point scalar math on partition 0 (DVE) ----
        # segment layout on [1, 384]: cols li*128 + (b*32 + n), n = s*4 + rb
        SEG = lambda t, li: t[:, li * 128:(li + 1) * 128]

        CS = pa.tile([1, 384], F32, name="CS")       # E-1 per level
        for li, (H, W) in enumerate(LEVELS):
            nc.vector.memset(SEG(CS, li), float(W - 1))
        CE2 = pa.tile([1, 384], F32, name="CE2")     # E-2
        nc.vector.tensor_scalar(
            out=CE2[:], in0=CS[:], scalar1=-1.0, scalar2=None, op0=AL.add
        )
        CW = pa.tile([1, 384], F32, name="CW")       # E (row stride)
        nc.vector.tensor_scalar(
            out=CW[:], in0=CS[:], scalar1=1.0, scalar2=None, op0=AL.add
        )
        OFF = pa.tile([1, 384], F32, name="OFF")     # (b % nb) * HW
        nc.vector.memset(SEG(OFF, 0), 0.0)           # l0: nb=1
        for li in (1, 2):
            HW = LEVELS[li][0] * LEVELS[li][1]
            ov = SEG(OFF, li).rearrange("o (b n) -> o b n", b=BL)
            for b in range(BL):
                nc.vector.memset(ov[:, b], float(b * HW))

        bxv = bxr[:].rearrange("o (j c) -> o j c", c=4)

        def axis_prep(coord, ax):
            """p=clip(c*(E-1),0,E-1); e0=clamp(floor(p),0,E-2); w=p-e0.
            floor via 16.16 fixed point (exact *2^16)."""
            pf = pa.tile([1, 384], F32, name=f"pf{ax}", tag="tmp_pf")
            for li in range(3):
                nc.vector.tensor_tensor(
                    out=SEG(pf, li), in0=coord, in1=SEG(CS, li), op=AL.mult
                )
            nc.vector.tensor_scalar_max(out=pf[:], in0=pf[:], scalar1=0.0)
            nc.vector.tensor_tensor(out=pf[:], in0=pf[:], in1=CS[:], op=AL.min)
            pxs = pa.tile([1, 384], F32, name=f"pxs{ax}", tag="tmp_pxs")
            nc.vector.tensor_scalar(
                out=pxs[:], in0=pf[:], scalar1=65536.0, scalar2=None, op0=AL.mult
            )
            ifx = pa.tile([1, 384], dt.int32, name=f"ifx{ax}", tag="tmp_ifx")
            nc.vector.tensor_copy(out=ifx[:], in_=pxs[:])
            nc.vector.tensor_scalar(
                out=ifx[:], in0=ifx[:], scalar1=16, scalar2=None,
                op0=AL.arith_shift_right,
            )
            e0 = pa.tile([1, 384], F32, name=f"e0{ax}")
            nc.vector.tensor_copy(out=e0[:], in_=ifx[:])
            nc.vector.tensor_tensor(out=e0[:], in0=e0[:], in1=CE2[:], op=AL.min)
            we = pa.tile([1, 384], F32, name=f"we{ax}")
            nc.vector.tensor_tensor(out=we[:], in0=pf[:], in1=e0[:], op=AL.subtract)
            return e0, we

        e0x, wx = axis_prep(bxv[:, :, 0], "x")
        e0y, wy = axis_prep(bxv[:, :, 1], "y")

        # basef[li, b, n] = y0*W + x0 + (b%nb)*HW  (exact small integers)
        basef = pa.tile([1, 384], F32, name="basef")
        nc.vector.tensor_tensor(out=basef[:], in0=e0y[:], in1=CW[:], op=AL.mult)
        nc.vector.tensor_tensor(out=basef[:], in0=basef[:], in1=e0x[:], op=AL.add)
        nc.vector.tensor_tensor(out=basef[:], in0=basef[:], in1=OFF[:], op=AL.add)
        basei = pa.tile([1, 384], I16, name="basei")
        nc.vector.tensor_copy(out=basei[:], in_=basef[:])

        # ---- srow: wrapped int16 gather indices, [16 rows, 96 q-cols] ----
        # gather out col = q*16 + r; index stored at wrapped [r, q].
        # l2: q0=0,  l1: q0=32  (d=2 parity scheme):
        #   q = sec*16 + b*4 + s_hi, r = s_lo*8 + rb*2 + row
        #   p = base + row*W;  idx = (p>>1) + (p&1)*(nb*HW/2) + sec*(nb*HW)
        # l0: q0=64 (d=1 4-corner, shared by both chunk tiles):
        #   q = u*8 + s, r = rb*4 + k;  idx = base + dk(k)
        QT = 96
        srow = pa.tile([1, 16 * QT], I16, name="srow")

        for li, q0 in ((2, 0), (1, 32)):
            H, W = LEVELS[li]
            nbHW = BL * H * W
            for row in range(2):
                prow = pa.tile([1, 128], dt.int32, name=f"prow{li}{row}", tag="prow")
                nc.vector.tensor_scalar(
                    out=prow[:], in0=SEG(basei, li), scalar1=row * W,
                    scalar2=None, op0=AL.add,
                )
                par = pa.tile([1, 128], dt.int32, name=f"par{li}{row}", tag="par")
                nc.vector.tensor_scalar(
                    out=par[:], in0=prow[:], scalar1=1, scalar2=None,
                    op0=AL.bitwise_and,
                )
                nc.vector.tensor_scalar(
                    out=par[:], in0=par[:], scalar1=nbHW // 2, scalar2=None,
                    op0=AL.mult,
                )
                nc.vector.tensor_scalar(
                    out=prow[:], in0=prow[:], scalar1=1, scalar2=None,
                    op0=AL.arith_shift_right,
                )
                nc.vector.tensor_tensor(
                    out=prow[:], in0=prow[:], in1=par[:], op=AL.add
                )
                # scatter into srow sec0 cols: out[o, rb, b, s_hi] at
                # flat = (s_lo*8 + rb*2 + row)*QT + q0 + b*4 + s_hi
                sv = srow[:].rearrange(
                    "o (sl rb2 row2 q) -> o sl rb2 row2 q",
                    sl=2, rb2=4, row2=2,
                )  # r = s_lo*8 + rb*2 + row
                pv = prow[:].rearrange(
                    "o (b sh sl f) -> o b sh sl f", b=BL, sh=4, sl=2
                )  # col = b*32 + s_hi*8 + s_lo*4 + rb
                for s_lo in range(2):
                    otv = sv[:, s_lo, :, row, q0:q0 + 16].rearrange(
                        "o rb (b s) -> o rb b s", b=BL
                    )
                    itv = pv[:, :, :, s_lo, :].rearrange(
                        "o b sh f -> o f b sh"
                    )
                    nc.vector.tensor_copy(out=otv, in_=itv)
            # sec=1 cols = sec=0 cols + nb*HW
            sq = srow[:].rearrange("o (r q) -> o r q", r=16)
            nc.vector.tensor_scalar(
                out=sq[:, :, q0 + 16:q0 + 32], in0=sq[:, :, q0:q0 + 16],
                scalar1=nbHW, scalar2=None, op0=AL.add,
            )

        # l0 (d=1): per (u, k): out[o, rb, s] at r=rb*4+k, q=64+u*8+s
        sq = srow[:].rearrange("o (rb k q) -> o rb k q", rb=4, k=4)
        b0 = SEG(basei, 0).rearrange("o (b s f) -> o f b s", b=BL, f=4)
        W0 = LEVELS[0][1]
        for u in range(BL):
            for k in range(4):
                dk = (k // 2) * W0 + (k % 2)
                nc.vector.tensor_scalar(
                    out=sq[:, :, k, 64 + u * 8:64 + u * 8 + 8],
                    in0=b0[:, :, u, :], scalar1=dk, scalar2=None, op0=AL.add,
                )

        # ---- replicate srow -> widx [128, 96] via DRAM round trip ----
        sidx_d = pdram.tile([16, QT], I16, name="sidx")
        nc.sync.dma_start(
            out=sidx_d[:], in_=srow[:].rearrange("o (r q) -> o r q", r=16)
        )
        widx = pool.tile([128, QT], I16, name="widx")
        nc.sync.dma_start(
            out=widx[:],
            in_=sidx_d[:].unsqueeze(0).broadcast_to([8, 16, QT]),
        )

        # ---- lerp weights wrow [1, 1536] -> wb [128, 1536] ----
        # l2: 0:512, l1: 512:1024  (cols (b, s, rb, row, j) = pt*4+row*2+j,
        #   weight = yw(row) * xw(j))
        # l0: 1024:1536 (cols (u, s, rb, k) = pt*4+k, weight = yw(k)*xw(k))
        w1x = pa.tile([1, 384], F32, name="w1x")
        nc.vector.tensor_scalar(
            out=w1x[:], in0=wx[:], scalar1=-1.0, scalar2=1.0,
            op0=AL.mult, op1=AL.add,
        )
        w1y = pa.tile([1, 384], F32, name="w1y")
        nc.vector.tensor_scalar(
            out=w1y[:], in0=wy[:], scalar1=-1.0, scalar2=1.0,
            op0=AL.mult, op1=AL.add,
        )
        wrow = pa.tile([1, 1536], F32, name="wrow")
        iv = lambda t, li: SEG(t, li).rearrange(
            "o (b s f) -> o b s f", b=BL, f=4
        )
        for li, w0 in ((2, 0), (1, 512)):
            wseg = wrow[:, w0:w0 + 512].rearrange(
                "o (b s f row j) -> o b s f row j", b=BL, s=8, f=4, row=2
            )
            for row, ywt in ((0, w1y), (1, wy)):
                for j, xwt in ((0, w1x), (1, wx)):
                    nc.vector.tensor_tensor(
                        out=wseg[:, :, :, :, row, j],
                        in0=iv(ywt, li), in1=iv(xwt, li), op=AL.mult,
                    )
        wseg = wrow[:, 1024:1536].rearrange(
            "o (b s f k) -> o b s f k", b=BL, s=8, f=4
        )
        for k, (ywt, xwt) in enumerate(
            [(w1y, w1x), (w1y, wx), (wy, w1x), (wy, wx)]
        ):
            nc.vector.tensor_tensor(
                out=wseg[:, :, :, :, k], in0=iv(ywt, 0), in1=iv(xwt, 0),
                op=AL.mult,
            )
        wrow_d = pdram.tile([1, 1536], F32, name="wrow_d")
        nc.sync.dma_start(out=wrow_d[:], in_=wrow[:])
        wb = pool.tile([128, 1536], F32, name="wb")
        nc.sync.dma_start(out=wb[:], in_=wrow_d[:].broadcast_to([128, 1536]))

        # ---- gathers + lerp + reduce + per-chunk channel sums ----
        V = pool.tile([128, 768], F32, name="V")
        ones = pool.tile([128, 1], F32)
        nc.vector.memset(ones[:], 1.0)

        ps_ss = ppsum.tile([1, 512], F32, name="ps_ss")    # ss2 | ss1
        ps_ss0 = ppsum.tile([1, 256], F32, name="ps_ss0")  # (u, sec, n)
        ps_d12 = ppsum.tile([1, 256], F32, name="ps_d12")
        ps_d01 = ppsum.tile([1, 256], F32, name="ps_d01")
        ps_d02 = ppsum.tile([1, 256], F32, name="ps_d02")

        def colsum(ps_slice, in0, in1, n, tag):
            prod = pwork.tile([128, n], F32, name=f"prod{tag}", tag=f"prod{n}",
                              bufs=1)
            nc.vector.tensor_tensor(out=prod[:], in0=in0, in1=in1, op=AL.mult)
            nc.tensor.matmul(ps_slice, ones[:], prod[:], start=True, stop=True)

        # l2 / l1: one d=2 gather each, og [128, 1024]
        for li, T, q0, v0, w0, tag in (
            (2, T2, 0, 0, 0, "2"), (1, T1, 32, 256, 512, "1"),
        ):
            og = pwork.tile([128, 1024], F32, name=f"og{tag}", tag="ogL",
                            bufs=1)
            nc.gpsimd.ap_gather(
                out_ap=og[:],
                in_ap=T[:].rearrange("c (n e) -> c n e", e=2),
                idxs_ap=widx[:, q0:q0 + 32],
                channels=128, num_elems=T.shape[1] // 2, d=2, num_idxs=512,
            )
            for sec in range(2):
                nc.vector.tensor_tensor(
                    out=og[:, sec * 512:(sec + 1) * 512],
                    in0=og[:, sec * 512:(sec + 1) * 512],
                    in1=wb[:, w0:w0 + 512], op=AL.mult,
                )
            nc.vector.tensor_reduce(
                out=V[:, v0:v0 + 256],
                in_=og[:].rearrange("c (n f) -> c n f", f=4),
                axis=mybir.AxisListType.X, op=AL.add,
            )
            colsum(ps_ss[:, v0:v0 + 256], V[:, v0:v0 + 256],
                   V[:, v0:v0 + 256], 256, f"ss{tag}")
            if li == 1:
                colsum(ps_d12[:], V[:, 256:512], V[:, 0:256], 256, "d12")

        # l0: 8 per-(image, chunk) d=1 gathers, og [128, 128]
        for u in range(BL):
            for sec in range(2):
                og = pwork.tile([128, 128], F32, name=f"og0{u}{sec}",
                                tag="og0", bufs=1)
                nc.gpsimd.ap_gather(
                    out_ap=og[:], in_ap=T0[2 * u + sec][:],
                    idxs_ap=widx[:, 64 + u * 8:64 + u * 8 + 8],
                    channels=128, num_elems=4096, d=1, num_idxs=128,
                )
                nc.vector.tensor_tensor(
                    out=og[:], in0=og[:],
                    in1=wb[:, 1024 + u * 128:1024 + (u + 1) * 128],
                    op=AL.mult,
                )
                v0 = 512 + u * 64 + sec * 32
                nc.vector.tensor_reduce(
                    out=V[:, v0:v0 + 32],
                    in_=og[:].rearrange("c (n f) -> c n f", f=4),
                    axis=mybir.AxisListType.X, op=AL.add,
                )
            # per-image channel sums once both chunks are in
            v0u = V[:, 512 + 64 * u:512 + 64 * (u + 1)]
            v1u = V[:, 256:512].rearrange(
                "c (sec b n) -> c sec b n", sec=2, b=BL
            )[:, :, u, :]
            v2u = V[:, 0:256].rearrange(
                "c (sec b n) -> c sec b n", sec=2, b=BL
            )[:, :, u, :]
            sl = slice(u * 64, (u + 1) * 64)
            colsum(ps_ss0[:, sl], v0u, v0u, 64, f"ss0{u}")
            colsum(ps_d01[:, sl], v0u, v1u, 64, f"d01{u}")
            colsum(ps_d02[:, sl], v0u, v2u, 64, f"d02{u}")

        # ---- epilogue on partition 0 ----
        cs_ss = pool.tile([1, 512], F32, name="cs_ss")
        nc.vector.tensor_copy(out=cs_ss[:], in_=ps_ss[:])
        cs_ss0 = pool.tile([1, 256], F32, name="cs_ss0")
        nc.vector.tensor_copy(out=cs_ss0[:], in_=ps_ss0[:])
        cs_d01 = pool.tile([1, 256], F32, name="cs_d01")
        nc.vector.tensor_copy(out=cs_d01[:], in_=ps_d01[:])
        cs_d02 = pool.tile([1, 256], F32, name="cs_d02")
        nc.vector.tensor_copy(out=cs_d02[:], in_=ps_d02[:])
        cs_d12 = pool.tile([1, 256], F32, name="cs_d12")
        nc.vector.tensor_copy(out=cs_d12[:], in_=ps_d12[:])

        ssc = pool.tile([1, 384], F32, name="ssc")
        dc = pool.tile([1, 384], F32, name="dc")

        def secsum(dst, src, l0_layout):
            if l0_layout:  # src [1, 256] cols (u, sec, n)
                v = src.rearrange("o (u sec n) -> o u sec n", u=BL, sec=2)
                nc.vector.tensor_tensor(
                    out=dst.rearrange("o (u n) -> o u n", u=BL),
                    in0=v[:, :, 0], in1=v[:, :, 1], op=AL.add,
                )
            else:  # src [1, 256] cols (sec, b, n)
                nc.vector.tensor_tensor(
                    out=dst, in0=src[:, 0:128], in1=src[:, 128:256], op=AL.add
                )

        secsum(SEG(ssc, 0), cs_ss0[:], True)
        secsum(SEG(ssc, 1), cs_ss[:, 256:512], False)
        secsum(SEG(ssc, 2), cs_ss[:, 0:256], False)
        secsum(SEG(dc, 0), cs_d01[:], True)
        secsum(SEG(dc, 1), cs_d02[:], True)
        secsum(SEG(dc, 2), cs_d12[:], False)

        # rn = 1 / max(sqrt(ssc), EPS) == 1 / sqrt(max(ssc, EPS^2))
        nc.vector.tensor_scalar_max(out=ssc[:], in0=ssc[:], scalar1=EPS * EPS)
        nrm = pool.tile([1, 384], F32, name="nrm")
        nc.scalar.sqrt(out=nrm[:], in_=ssc[:])
        rn = pool.tile([1, 384], F32, name="rn")
        nc.vector.reciprocal(out=rn[:], in_=nrm[:])

        rp = pool.tile([1, 384], F32, name="rp")
        for seg, (i, j) in enumerate(PAIRS):
            nc.vector.tensor_tensor(
                out=SEG(rp, seg), in0=SEG(rn, i), in1=SEG(rn, j), op=AL.mult
            )
        nc.vector.tensor_tensor(out=dc[:], in0=dc[:], in1=rp[:], op=AL.mult)
        res = pool.tile([1, 1], F32)
        nc.vector.tensor_reduce(
            out=res[:], in_=dc[:], axis=mybir.AxisListType.X, op=AL.add
        )
        nc.sync.dma_start(out=out.ap(), in_=res[:])

    nc.compile()
    return nc


def _get_program():
    if "nc" not in _CACHE:
        _CACHE["nc"] = _build_program()
    return _CACHE["nc"]


def _run_device(feat0, feat1, feat2, boxes, **run_kwargs):
    from concourse.bass_utils import run_bass_kernel_spmd

    nc = _get_program()

    feats = [
        np.ascontiguousarray(np.asarray(f, dtype=np.float32))
        for f in (feat0, feat1, feat2)
    ]
    boxes = np.ascontiguousarray(np.asarray(boxes, dtype=np.float32))

    in_maps = []
    for k in range(N_CORES):
        sl = slice(k * BL, (k + 1) * BL)
        in_maps.append(
            {
                "feat0": feats[0][sl],
                "feat1": feats[1][sl],
                "feat2": feats[2][sl],
                "boxes": boxes[sl],
            }
        )

    return run_bass_kernel_spmd(
        nc, in_maps, core_ids=list(range(N_CORES)), **run_kwargs
    )


def kernel(feat0, feat1, feat2, boxes):
    r = _run_device(feat0, feat1, feat2, boxes)
    total = np.float64(0.0)
    for m in r.results:
        total += np.float64(m["out"].reshape(-1)[0])

    count = B * N * len(PAIRS)
    avg = np.float32(total) / np.float32(count)
    loss = np.float32(1.0) - avg
    loss = np.nan_to_num(loss, nan=0.0, posinf=1.0, neginf=0.0)
    return np.array(np.clip(loss, 0.0, 2.0), dtype=np.float32)
